# revision 1
# baseline (speedup 1.0000x reference)
"""Sparse attention kernel for Trainium2 (8 NeuronCores, data-parallel over batch).

Reference computation (per batch row b):
    q    = x @ q_w.T                                  [N, C]
    xkv  = x[key_ind]                                 [NKV, C]
    kv   = xkv @ kv_w.T -> per-head k, v              [NKV, 2C]
    attn = softmax((q*scale) @ k.T) @ v               [N, C]
    out  = attn @ proj_w.T + proj_b                   [N, C]

Per-core layout strategy (core = one batch row):
  - Everything computed transposed ("feature on partition"): qT [C, N],
    kT/vT via kv_w.T, attention scores ST [m, n] so that softmax needs no
    transposes: P = exp(ST) is directly the PV stationary operand, and the
    softmax denominator falls out of an appended ones-column in v.
  - f32r (TF32-like) matmuls throughout: full PE rate at ~1e-3 rel err.
  - KV gather on device via indirect DMA + PE transposes.
"""
import os
import sys

sys.path.insert(0, "/opt/trn_rl_repo")

STAGE = int(os.environ.get("BK_STAGE", "99"))

import numpy as np  # noqa: E402

B, N, C = 8, 2048, 768
NKV = 512
H = 12
HD = C // H          # 64
SCALE = HD ** -0.5
P = 128
CT = C // P          # 6 feature tiles
NC2 = 512            # token chunk
NCH = N // NC2       # 4 chunks
MCH = NKV // P       # 4 key chunks
G = H // 2           # 6 head pairs

_CACHE = {}


def _build():
    import concourse.bass as bass
    import concourse.mybir as mybir
    import concourse.tile as tile
    from concourse import bacc
    from concourse.masks import make_identity
    from contextlib import ExitStack

    F32 = mybir.dt.float32
    F32R = mybir.dt.float32r
    I32 = mybir.dt.int32
    Exp = mybir.ActivationFunctionType.Exp
    Ident = mybir.ActivationFunctionType.Identity

    nc = bacc.Bacc("TRN2", target_bir_lowering=False, debug=False, num_devices=8)

    xT = nc.dram_tensor("xT", [CT, P, N], F32R, kind="ExternalInput")
    xr = nc.dram_tensor("xr", [N, C], F32, kind="ExternalInput")
    idx = nc.dram_tensor("idx", [P, MCH], I32, kind="ExternalInput")
    qwT = nc.dram_tensor("qwT", [CT, P, C], F32R, kind="ExternalInput")
    kwT = nc.dram_tensor("kwT", [CT, P, C], F32R, kind="ExternalInput")
    vwT = nc.dram_tensor("vwT", [CT, P, C], F32R, kind="ExternalInput")
    ones_in = nc.dram_tensor("ones_in", [P, HD], F32R, kind="ExternalInput")
    pwT = nc.dram_tensor("pwT", [CT, P, C], F32R, kind="ExternalInput")
    pb = nc.dram_tensor("pb", [P, CT], F32, kind="ExternalInput")
    out = nc.dram_tensor("out", [CT, P, N], F32, kind="ExternalOutput")

    with tile.TileContext(nc) as tc, ExitStack() as top:
        const = top.enter_context(tc.tile_pool(name="const", bufs=1))
        work = top.enter_context(tc.tile_pool(name="work", bufs=3))
        apool = top.enter_context(tc.tile_pool(name="apool", bufs=1))
        w3 = top.enter_context(tc.tile_pool(name="w3", bufs=2))
        ptp = top.enter_context(tc.tile_pool(name="ptp", bufs=5))
        rcrb = top.enter_context(tc.tile_pool(name="rcrb", bufs=1))
        w4 = top.enter_context(tc.tile_pool(name="w4", bufs=3))

        # ---------- persistent loads ----------
        idx_sb = const.tile([P, MCH], I32, tag="idx")
        nc.sync.dma_start(idx_sb[:], idx[:])
        qwT_sb = []
        kwT_sb = []
        vwT_sb = []
        pwT_sb = []
        pb_sb = const.tile([P, CT], F32, tag="pb")
        nc.sync.dma_start(pb_sb[:], pb[:])

        # ---------- gather + transpose + KV projection ----------
        kT_sb = []      # per head pair g: [128, NKV], rows 0-63 head 2g, 64-127 head 2g+1
        vaug_sb = []    # per m-chunk: [128, H*(HD+1)] f32r, col HD of each head block = 1.0
        with ExitStack() as ph:
            gpool = ph.enter_context(tc.tile_pool(name="gather", bufs=1))
            ps_tr = ph.enter_context(tc.tile_pool(name="ps_tr", bufs=4, space="PSUM"))
            ps_kv = ph.enter_context(tc.tile_pool(name="ps_kv", bufs=4, space="PSUM"))

            for i in range(CT):
                t = gpool.tile([P, C], F32R, tag=f"kwT{i}")
                nc.sync.dma_start(t[:], kwT[i, :, :])
                kwT_sb.append(t)
                t = gpool.tile([P, C], F32R, tag=f"vwT{i}")
                nc.sync.dma_start(t[:], vwT[i, :, :])
                vwT_sb.append(t)
            xkvT = []
            for i in range(CT):
                xkvT_i = gpool.tile([P, NKV], F32R, tag=f"xkvT{i}")
                xkvT.append(xkvT_i)
            xkv_tiles = []
            gather_insts = []
            for k in range(MCH):
                xkv = gpool.tile([P, C], F32, tag=f"xkv{k % 2}")
                gi = nc.gpsimd.indirect_dma_start(
                    out=xkv[:], out_offset=None, in_=xr[:],
                    in_offset=bass.IndirectOffsetOnAxis(ap=idx_sb[:, k:k + 1], axis=0))
                gather_insts.append(gi)
                xkv_tiles.append(xkv)
            ident = const.tile([P, P], F32, tag="ident")
            make_identity(nc, ident[:])
            for k in range(MCH):
                xkv = xkv_tiles[k]
                for i in range(CT):
                    tr = ps_tr.tile([P, P], F32, tag="tr")
                    nc.tensor.transpose(tr[:], xkv[:, i * P:(i + 1) * P], ident[:])
                    nc.scalar.copy(xkvT[i][:, k * P:(k + 1) * P], tr[:])

            # kT: per head pair
            for g in range(G):
                kp = ps_kv.tile([P, NKV], F32, tag="kv")
                for i in range(CT):
                    nc.tensor.matmul(kp[:], kwT_sb[i][:, g * P:(g + 1) * P], xkvT[i][:],
                                     start=(i == 0), stop=(i == CT - 1))
                kt = const.tile([P, NKV], F32R, tag=f"kT{g}")
                nc.vector.tensor_copy(kt[:], kp[:])
                kT_sb.append(kt)

            # v (+ ones col): [m, head*(HD+1)]
            for k in range(MCH):
                va = const.tile([P, H * (HD + 1)], F32R, tag=f"vaug{k}")
                va3 = va[:].rearrange("p (h x) -> p h x", x=HD + 1)
                for half in range(2):
                    vp = ps_kv.tile([P, 6 * HD], F32, tag="kv")
                    for i in range(CT):
                        nc.tensor.matmul(vp[:], xkvT[i][:, k * P:(k + 1) * P],
                                         vwT_sb[i][:, half * 6 * HD:(half + 1) * 6 * HD],
                                         start=(i == 0), stop=(i == CT - 1))
                    nc.scalar.copy(va3[:, 6 * half:6 * half + 6, 0:HD],
                                   vp[:].rearrange("p (h x) -> p h x", x=HD))
                nc.sync.dma_start(va3[:, :, HD:HD + 1], ones_in[:, 0:H])
                vaug_sb.append(va)

        # late weight loads (q/proj not needed until after gather/kv phase)
        for i in range(CT):
            t = const.tile([P, C], F32R, tag=f"qwT{i}")
            nc.sync.dma_start(t[:], qwT[i, :, :])
            qwT_sb.append(t)
            t = const.tile([P, C], F32R, tag=f"pwT{i}")
            nc.sync.dma_start(t[:], pwT[i, :, :])
            pwT_sb.append(t)

        # ---------- main loop over token chunks ----------
        if STAGE <= 1:
            z = work.tile([P, CT * NC2], F32, tag="outc")
            nc.gpsimd.memset(z[:], 0.0)
            for ch in range(NCH):
                sl = slice(ch * NC2, (ch + 1) * NC2)
                nc.sync.dma_start(out[:, :, sl].rearrange("i p n -> p i n"),
                                  z[:].rearrange("p (i n) -> p i n", i=CT))
            nc.compile()
            return nc
        ps_mm = top.enter_context(tc.tile_pool(name="ps_mm", bufs=2, space="PSUM"))
        ps_st = top.enter_context(tc.tile_pool(name="ps_st", bufs=2, space="PSUM"))
        ps_ov = top.enter_context(tc.tile_pool(name="ps_ov", bufs=2, space="PSUM"))
        for ch in range(NCH):
            sl = slice(ch * NC2, (ch + 1) * NC2)
            xTc = []
            for i in range(CT):
                t = w4.tile([P, NC2], F32R, tag=f"xTc{i}")
                nc.sync.dma_start(t[:], xT[i, :, sl])
                xTc.append(t)

            # qT for this chunk: per head pair g -> [128, NC2]
            qT = []
            for j in range(CT):
                qp = ps_mm.tile([P, NC2], F32, tag="mm")
                for i in range(CT):
                    nc.tensor.matmul(qp[:], qwT_sb[i][:, j * P:(j + 1) * P], xTc[i][:],
                                     start=(i == 0), stop=(i == CT - 1))
                qt = w3.tile([P, NC2], F32R, tag=f"qT{j}")
                nc.vector.tensor_copy(qt[:], qp[:])
                qT.append(qt)

            # attention per head; output packed per head pair:
            # attn[g] [128, NC2], rows 0-63 = head 2g, rows 64-127 = head 2g+1
            attn = []
            for g in range(G):
                at = apool.tile([P, NC2], F32R, tag=f"attn{g}")
                # all 8 STs of the pair adjacent: T0/T8 row-group streams
                # overlap on the PE and the 64-row mode is entered once
                pts2 = {}
                for par in range(2):
                    base = par * HD
                    for k in range(MCH):
                        st = ps_st.tile([P, NC2], F32, tag="st")
                        nc.tensor.matmul(
                            st[:],
                            kT_sb[g][base:base + HD, k * P:(k + 1) * P],
                            qT[g][base:base + HD, :],
                            start=True, stop=True)
                        pt = ptp.tile([P, NC2], F32R, tag="pt")
                        nc.scalar.activation(pt[:], st[:], Exp, scale=SCALE)
                        pts2[(par, k)] = pt
                for par in range(2):
                    h = 2 * g + par
                    base = par * HD
                    ov = ps_ov.tile([HD + 1, NC2], F32, tag="ov")
                    for k in range(MCH):
                        nc.tensor.matmul(ov[:], vaug_sb[k][:, h * (HD + 1):(h + 1) * (HD + 1)],
                                         pts2[(par, k)][:], start=(k == 0),
                                         stop=(k == MCH - 1))
                    rc = rcrb.tile([1, NC2], F32, tag="rc")
                    nc.vector.reciprocal(rc[:], ov[HD:HD + 1, :])
                    rb = rcrb.tile([HD, NC2], F32, tag="rb")
                    nc.gpsimd.partition_broadcast(rb[:], rc[:])
                    nc.vector.tensor_mul(at[base:base + HD, :], ov[0:HD, :], rb[:])
                attn.append(at)

            # output projection + bias (plain K=128 over head pairs)
            for j in range(CT):
                pp = ps_mm.tile([P, NC2], F32, tag="mmp")
                for i in range(CT):
                    nc.tensor.matmul(
                        pp[:], pwT_sb[i][:, j * P:(j + 1) * P], attn[i][:],
                        start=(i == 0), stop=(i == CT - 1))
                oj = work.tile([P, NC2], F32, tag="oj")
                nc.vector.tensor_scalar_add(oj[:], pp[:], pb_sb[:, j:j + 1])
                nc.sync.dma_start(out[j, :, sl], oj[:])

    nc.compile()
    return nc


def _get_nc():
    if "nc" not in _CACHE:
        _CACHE["nc"] = _build()
    return _CACHE["nc"]


def _prep_core_inputs(x, key_ind, q_w, kv_w, proj_w, proj_b):
    """Build the 8 per-core input maps."""
    x = np.ascontiguousarray(x, dtype=np.float32)
    qwT = np.ascontiguousarray(q_w.T.astype(np.float32).reshape(CT, P, C))
    kvwT = kv_w.T.astype(np.float32)                       # [C, 2C]
    kvwT3 = kvwT.reshape(C, H, 2 * HD)
    kwT = np.ascontiguousarray(kvwT3[:, :, :HD].reshape(CT, P, C))
    vwT = np.ascontiguousarray(kvwT3[:, :, HD:].reshape(CT, P, C))
    ones_hd = np.ones((P, HD), dtype=np.float32)
    pwT = np.ascontiguousarray(proj_w.T.astype(np.float32).reshape(CT, P, C))
    pb = np.ascontiguousarray(proj_b.astype(np.float32).reshape(CT, P).T)
    in_maps = []
    for b in range(B):
        xb = x[b]                                   # [N, C]
        xTb = np.ascontiguousarray(xb.T.reshape(CT, P, N))
        idxb = np.ascontiguousarray(
            key_ind[b].astype(np.int32).reshape(MCH, P).T)
        in_maps.append({
            "xT": xTb, "xr": xb, "idx": idxb,
            "qwT": qwT, "kwT": kwT, "vwT": vwT, "pwT": pwT, "pb": pb,
            "ones_in": ones_hd,
        })
    return in_maps


def kernel(x, key_ind, q_w, kv_w, proj_w, proj_b, _trace=False, _results=None):
    from concourse.bass_utils import run_bass_kernel_spmd

    nc = _get_nc()
    in_maps = _prep_core_inputs(x, key_ind, q_w, kv_w, proj_w, proj_b)
    res = run_bass_kernel_spmd(nc, in_maps, core_ids=list(range(B)), trace=_trace)
    if _results is not None:
        _results.append(res)
    out = np.empty((B, N, C), dtype=np.float32)
    for b in range(B):
        out[b] = res.results[b]["out"].reshape(C, N).T
    return out



# revision 43
# speedup vs baseline: 1.1457x; 1.1457x over previous
"""Sparse attention kernel for Trainium2 (8 NeuronCores, data-parallel over batch).

Reference computation (per batch row b):
    q    = x @ q_w.T                                  [N, C]
    xkv  = x[key_ind]                                 [NKV, C]
    kv   = xkv @ kv_w.T -> per-head k, v              [NKV, 2C]
    attn = softmax((q*scale) @ k.T) @ v               [N, C]
    out  = attn @ proj_w.T + proj_b                   [N, C]

Per-core layout (core = one batch row), all-bf16 matmul dataflow:
  - q/k computed feature-major (qT/kT [feat, tok]); scores ST [m, n] in PSUM;
    P = exp(ST*scale) written bf16 to SBUF by the Act engine.
  - PV is "flipped": stationary = P [m, n-128], moving = v_aug [m, 65]
    (65th column of ones gives the softmax denominator for free), so the
    PSUM output [n, 65] uses full 128 output partitions per matmul row.
  - attn normalized token-major (one broadcast multiply per 6 heads), then
    PE-transposed back to feature-major for the output projection.
  - Weights land as one large DMA each ([P, CT*C] layout prepped on host),
    spread across the SP and Act DMA queues; warmup matmuls burn the PE
    p-state ramp while they arrive.
"""
import sys

sys.path.insert(0, "/opt/trn_rl_repo")

import numpy as np  # noqa: E402

B, N, C = 8, 2048, 768
NKV = 512
H = 12
HD = C // H          # 64
SCALE = HD ** -0.5
P = 128
CT = C // P          # 6 feature tiles
NC2 = 512            # token chunk
NCH = N // NC2       # 4 chunks
MCH = NKV // P       # 4 key chunks
G = H // 2           # 6 head pairs

_CACHE = {}


def _build():
    import concourse.bass as bass
    import concourse.mybir as mybir
    import concourse.tile as tile
    from concourse import bacc
    from concourse.masks import make_identity
    from contextlib import ExitStack

    F32 = mybir.dt.float32
    BF = mybir.dt.bfloat16
    I32 = mybir.dt.int32
    Exp = mybir.ActivationFunctionType.Exp

    nc = bacc.Bacc("TRN2", target_bir_lowering=False, debug=False, num_devices=8)

    xT = nc.dram_tensor("xT", [P, CT, N], BF, kind="ExternalInput")
    xbf = nc.dram_tensor("xbf", [N, C], BF, kind="ExternalInput")
    idx = nc.dram_tensor("idx", [P, MCH], I32, kind="ExternalInput")
    qwT = nc.dram_tensor("qwT", [P, CT * C], BF, kind="ExternalInput")
    kwT = nc.dram_tensor("kwT", [P, CT * C], BF, kind="ExternalInput")
    vwT = nc.dram_tensor("vwT", [P, CT * C], BF, kind="ExternalInput")
    pwT = nc.dram_tensor("pwT", [P, CT * C], BF, kind="ExternalInput")
    pb = nc.dram_tensor("pb", [P, CT], F32, kind="ExternalInput")
    out = nc.dram_tensor("out", [CT, P, N], BF, kind="ExternalOutput")

    with tile.TileContext(nc) as tc, ExitStack() as top:
        const = top.enter_context(tc.tile_pool(name="const", bufs=1))
        xpool = top.enter_context(tc.tile_pool(name="xpool", bufs=1))
        qpool = top.enter_context(tc.tile_pool(name="qpool", bufs=2))
        ppool = top.enter_context(tc.tile_pool(name="ppool", bufs=2))
        npool = top.enter_context(tc.tile_pool(name="npool", bufs=1))
        tpool = top.enter_context(tc.tile_pool(name="tpool", bufs=1))
        opool = top.enter_context(tc.tile_pool(name="opool", bufs=1))
        rpool = top.enter_context(tc.tile_pool(name="rpool", bufs=2))

        # early PSUM pools: fresh banks so q/scores start during the prologue
        ps_q = top.enter_context(tc.tile_pool(name="ps_q", bufs=1, space="PSUM"))
        ps_st = top.enter_context(tc.tile_pool(name="ps_st", bufs=2, space="PSUM"))

        # ---------- warmup: burn the PE p-state ramp on dummies ----------
        zw = const.tile([P, NC2], BF, tag="zw")
        nc.vector.memset(zw[:], 0.0)
        wps = ps_q.tile([P, NC2], F32, tag="qp", name="wps")
        for w in range(18):
            nc.tensor.matmul(wps[:], zw[:, 0:P], zw[:],
                             start=(w == 0), stop=(w == 17))

        # ---------- DMAs: one large transfer per tensor, criticality order ----
        ph = top.enter_context(ExitStack())
        gpool = ph.enter_context(tc.tile_pool(name="kvph", bufs=1))
        idx_sb = const.tile([P, MCH], I32, tag="idx")
        nc.sync.dma_start(idx_sb[:], idx[:])
        xTc = {}
        t = xpool.tile([P, CT * NC2], BF, tag="xTc", name="xTc0")
        nc.sync.dma_start(t[:].rearrange("p (i n) -> p i n", n=NC2),
                          xT[:, :, 0:NC2])
        xTc[0] = t
        qwTb = const.tile([P, CT * C], BF, tag="qwTb")
        nc.sync.dma_start(qwTb[:], qwT[:])
        xkv_tiles = []
        for k in range(MCH):
            xkv = gpool.tile([P, C], BF, tag=f"xkv{k}")
            nc.gpsimd.indirect_dma_start(
                out=xkv[:], out_offset=None, in_=xbf[:],
                in_offset=bass.IndirectOffsetOnAxis(ap=idx_sb[:, k:k + 1], axis=0))
            xkv_tiles.append(xkv)
            if k == 0:
                # squeezed between gathers: ready before the first transpose
                ident = const.tile([P, P], BF, tag="ident")
                make_identity(nc, ident[:])
        # remaining inputs issue on the Pool queue with virtual timestamps
        # so the scheduler orders their descriptor-gens AFTER the gathers;
        # the in-order Pool queue then keeps their transfers behind them
        kwTb = gpool.tile([P, CT * C], BF, tag="kwTb")
        with tc.tile_wait_until(0.008):
            nc.gpsimd.dma_start(kwTb[:], kwT[:])
        vwTb = gpool.tile([P, CT * C], BF, tag="vwTb")
        with tc.tile_wait_until(0.010):
            nc.gpsimd.dma_start(vwTb[:], vwT[:])
        pwTb = const.tile([P, CT * C], BF, tag="pwTb")
        pb_sb = const.tile([P, CT], F32, tag="pb")
        with tc.tile_wait_until(0.013):
            nc.gpsimd.dma_start(pwTb[:], pwT[:])
            nc.gpsimd.dma_start(pb_sb[:], pb[:])

        qT = {}

        def emit_q(ch, j):
            qp = ps_q.tile([P, NC2], F32, tag="qp")
            for i in range(CT):
                nc.tensor.matmul(qp[:], qwTb[:, i * C + j * P:i * C + (j + 1) * P],
                                 xTc[ch][:, i * NC2:(i + 1) * NC2],
                                 start=(i == 0), stop=(i == CT - 1))
            qt = qpool.tile([P, NC2], BF, tag=f"qT{j}")
            nc.vector.tensor_copy(qt[:], qp[:])
            qT[(ch, j)] = qt

        # q for chunk 0 is the PE's bridge work while the gathers land
        for j in range(CT):
            emit_q(0, j)

        tc.tile_set_cur_wait(0.011)

        # ---------- KV phase: k-staggered so PE work starts per gather ------
        ps_ktr = ph.enter_context(tc.tile_pool(name="ps_ktr", bufs=1, space="PSUM"))
        ps_kv = ph.enter_context(tc.tile_pool(name="ps_kv", bufs=2, space="PSUM"))

        xkvT = gpool.tile([P, CT * NKV], BF, tag="xkvT")
        xkvT3 = xkvT[:].rearrange("p (i m) -> p i m", m=NKV)
        kTb = const.tile([P, G * NKV], BF, tag="kTb")
        kTb3 = kTb[:].rearrange("p (g m) -> p g m", m=NKV)
        for k in range(MCH):
            xkv = xkv_tiles[k]
            tr1 = ps_ktr.tile([P, NC2], BF, tag="ktr")
            for i in range(4):
                nc.tensor.transpose(tr1[:, i * P:(i + 1) * P],
                                    xkv[:, i * P:(i + 1) * P], ident[:])
            tr2 = ps_ktr.tile([P, NC2], BF, tag="ktr")
            for i in range(4, CT):
                nc.tensor.transpose(tr2[:, (i - 4) * P:(i - 3) * P],
                                    xkv[:, i * P:(i + 1) * P], ident[:])
            nc.vector.tensor_copy(
                xkvT3[:, 0:4, k * P:(k + 1) * P],
                tr1[:].rearrange("p (i m) -> p i m", m=P))
            nc.vector.tensor_copy(
                xkvT3[:, 4:CT, k * P:(k + 1) * P],
                tr2[:, 0:2 * P].rearrange("p (i m) -> p i m", m=P))
            # kT for this key block: per pair g a [128-feat, 128-key] region,
            # accumulated over the 6 input-feature tiles
            kpA = ps_kv.tile([P, NC2], F32, tag="kv")
            kpB = ps_kv.tile([P, 2 * P], F32, tag="kv2", bufs=1)
            for g in range(G):
                sl = kpA[:, g * P:(g + 1) * P] if g < 4 else \
                    kpB[:, (g - 4) * P:(g - 3) * P]
                for i in range(CT):
                    nc.tensor.matmul(
                        sl, kwTb[:, i * C + g * P:i * C + (g + 1) * P],
                        xkvT[:, i * NKV + k * P:i * NKV + (k + 1) * P],
                        start=(i == 0), stop=(i == CT - 1))
            nc.vector.tensor_copy(kTb3[:, 0:4, k * P:(k + 1) * P],
                                  kpA[:].rearrange("p (g m) -> p g m", m=P))
            nc.vector.tensor_copy(kTb3[:, 4:CT, k * P:(k + 1) * P],
                                  kpB[:].rearrange("p (g m) -> p g m", m=P))

        # vaug tiles: [m, H*(HD+1)] bf16, col HD of each head block stays 1.0
        vaug_sb = []
        for k in range(MCH):
            va = const.tile([P, H * (HD + 1)], BF, tag=f"vaug{k}")
            nc.vector.memset(va[:], 1.0)
            vaug_sb.append(va)

        # v: per m-chunk, token-major [m, feat] via stationary xkvT
        for k in range(MCH):
            va3 = vaug_sb[k][:].rearrange("p (h x) -> p h x", x=HD + 1)
            for half in range(2):
                vp = ps_kv.tile([P, NC2], F32, tag="kv")
                for i in range(CT):
                    nc.tensor.matmul(
                        vp[:, 0:6 * HD],
                        xkvT[:, i * NKV + k * P:i * NKV + (k + 1) * P],
                        vwTb[:, i * C + half * 6 * HD:i * C + (half + 1) * 6 * HD],
                        start=(i == 0), stop=(i == CT - 1))
                nc.vector.tensor_copy(
                    va3[:, 6 * half:6 * half + 6, 0:HD],
                    vp[:, 0:6 * HD].rearrange("p (h x) -> p h x", x=HD))
        tc.tile_set_cur_wait(0.0)
        ph.close()

        # ---------- late PSUM pools (reuse released prologue banks) ----------
        ps_pv = top.enter_context(tc.tile_pool(name="ps_pv", bufs=2, space="PSUM"))
        ps_pj = top.enter_context(tc.tile_pool(name="ps_pj", bufs=1, space="PSUM"))

        # ---------- main loop over token chunks ----------
        Pt = {}

        def emit_head_A(ch, h):
            g, par = h // 2, h % 2
            b0 = par * HD
            for k in range(MCH):
                st = ps_st.tile([P, NC2], F32, tag="st")
                nc.tensor.matmul(
                    st[:], kTb[b0:b0 + HD, g * NKV + k * P:g * NKV + (k + 1) * P],
                    qT[(ch, g)][b0:b0 + HD, :], start=True, stop=True)
                pt = ppool.tile([P, NC2], BF, tag=f"P{h}_{k}")
                nc.scalar.activation(pt[:], st[:], Exp, scale=SCALE)
                Pt[(ch, h, k)] = pt

        # chunk 0's scores run standalone (nothing to overlap with yet)
        for h in range(H):
            emit_head_A(0, h)

        for ch in range(NCH):
            base_n = ch * NC2
            # phase B: flipped PV + normalization + transpose + projection,
            # with the NEXT chunk's q chains, scores, and exps woven in so
            # the Act engine never starves while the PE works through B
            attnN = {}
            for s in range(4):
                attnN[s] = npool.tile([P, C], BF, tag=f"attnN{s}", name=f"attnN{s}")
            ojb = opool.tile([P, CT * NC2], BF, tag="ojb")
            ojb3 = ojb[:].rearrange("p (j n) -> p j n", n=NC2)

            seg = 0
            for pair in range(2):
                sset = (2 * pair, 2 * pair + 1)
                atTd = {}
                trd = {}
                for s in sset:
                    atTd[s] = tpool.tile([P, C], BF, tag=f"attnT{s}",
                                         name=f"atT{s}")
                # 2-head PV groups: group j's PV can start right after head
                # 2j+1's exp, and feeds transpose j immediately
                for grp in range(CT):
                    if ch + 1 < NCH:
                        if seg == 0:
                            t = xpool.tile([P, CT * NC2], BF, tag="xTc",
                                           name=f"xTc{ch + 1}")
                            nc.sync.dma_start(
                                t[:].rearrange("p (i n) -> p i n", n=NC2),
                                xT[:, :, (ch + 1) * NC2:(ch + 2) * NC2])
                            xTc[ch + 1] = t
                        if seg % 2 == 0:
                            emit_q(ch + 1, seg // 2)
                        emit_head_A(ch + 1, seg)
                    seg += 1
                    for s in sset:
                        pvt = ps_pv.tile([P, 2 * (HD + 1)], F32, tag="pv")
                        for h2 in range(2):
                            h = 2 * grp + h2
                            for k in range(MCH):
                                nc.tensor.matmul(
                                    pvt[:, h2 * (HD + 1):(h2 + 1) * (HD + 1)],
                                    Pt[(ch, h, k)][:, s * P:(s + 1) * P],
                                    vaug_sb[k][:, h * (HD + 1):(h + 1) * (HD + 1)],
                                    start=(k == 0), stop=(k == MCH - 1))
                        # normalization: 1/denominator, then broadcast mult
                        pv3 = pvt[:].rearrange("p (h x) -> p h x", x=HD + 1)
                        rc = rpool.tile([P, 2], F32, tag="rc")
                        rc3 = rc[:].rearrange("p (h x) -> p h x", x=1)
                        nc.vector.reciprocal(rc3, pv3[:, :, HD:HD + 1])
                        attnN3 = attnN[s][:].rearrange("p (h x) -> p h x", x=HD)
                        a0, a1 = bass.broadcast_tensor_aps(pv3[:, :, 0:HD], rc3)
                        nc.vector.tensor_tensor(
                            out=attnN3[:, 2 * grp:2 * grp + 2, :],
                            in0=a0, in1=a1, op=mybir.AluOpType.mult)
                        if grp == 0:
                            trd[s] = ps_st.tile([P, 3 * P], BF, tag="tr",
                                                bufs=1, name=f"tr1_{s}")
                        elif grp == 3:
                            trd[s] = ps_st.tile([P, 3 * P], BF, tag="tr",
                                                bufs=1, name=f"tr2_{s}")
                        trt = trd[s]
                        nc.tensor.transpose(trt[:, (grp % 3) * P:(grp % 3 + 1) * P],
                                            attnN[s][:, grp * P:(grp + 1) * P],
                                            ident[:])
                        if grp == 2:
                            nc.vector.tensor_copy(atTd[s][:, 0:3 * P], trt[:])
                        elif grp == 5:
                            nc.vector.tensor_copy(atTd[s][:, 3 * P:C], trt[:])
                # proj per subtile
                for s in sset:
                    atT = atTd[s]
                    pjA = ps_pj.tile([P, NC2], F32, tag="pjA")
                    pjB = ps_pj.tile([P, 2 * P], F32, tag="pjB")
                    for j in range(CT):
                        sl = pjA[:, j * P:(j + 1) * P] if j < 4 else \
                            pjB[:, (j - 4) * P:(j - 3) * P]
                        for i in range(CT):
                            nc.tensor.matmul(sl,
                                             pwTb[:, i * C + j * P:i * C + (j + 1) * P],
                                             atT[:, i * P:(i + 1) * P],
                                             start=(i == 0), stop=(i == CT - 1))
                    # batched bias add: psum -> oj staging, bias broadcast per j
                    pjA3 = pjA[:].rearrange("p (j n) -> p j n", n=P)
                    pbA = pb_sb[:, 0:4].rearrange("p (j x) -> p j x", x=1)
                    b0, b1 = bass.broadcast_tensor_aps(pjA3, pbA)
                    nc.vector.tensor_tensor(
                        out=ojb3[:, 0:4, s * P:(s + 1) * P],
                        in0=b0, in1=b1, op=mybir.AluOpType.add)
                    pjB3 = pjB[:].rearrange("p (j n) -> p j n", n=P)
                    pbB = pb_sb[:, 4:CT].rearrange("p (j x) -> p j x", x=1)
                    c0, c1 = bass.broadcast_tensor_aps(pjB3, pbB)
                    nc.vector.tensor_tensor(
                        out=ojb3[:, 4:CT, s * P:(s + 1) * P],
                        in0=c0, in1=c1, op=mybir.AluOpType.add)
                # drain this half-chunk's output; the very last one goes in
                # two j-half pieces so the tail transfer is half as long
                hb = pair * 2 * P
                if ch == NCH - 1 and pair == 1:
                    for jh in range(2):
                        nc.sync.dma_start(
                            out[3 * jh:3 * jh + 3, :,
                                base_n + hb:base_n + hb + 2 * P].rearrange(
                                "j p n -> p j n"),
                            ojb3[:, 3 * jh:3 * jh + 3, hb:hb + 2 * P])
                else:
                    nc.sync.dma_start(
                        out[:, :, base_n + hb:base_n + hb + 2 * P].rearrange(
                            "j p n -> p j n"),
                        ojb3[:, :, hb:hb + 2 * P])

    nc.compile()
    return nc


def _get_nc():
    if "nc" not in _CACHE:
        _CACHE["nc"] = _build()
    return _CACHE["nc"]


def _wprep(w):
    """[C_out, C_in] weight -> transposed, partition-major [P, CT*C]."""
    import ml_dtypes
    wt = np.ascontiguousarray(
        w.T.astype(np.float32).reshape(CT, P, C).transpose(1, 0, 2).reshape(P, CT * C))
    return wt.astype(ml_dtypes.bfloat16)


def _prep_core_inputs(x, key_ind, q_w, kv_w, proj_w, proj_b):
    """Build the 8 per-core input maps (host-side prep, all bf16)."""
    import ml_dtypes
    bf = ml_dtypes.bfloat16
    x = np.ascontiguousarray(x, dtype=np.float32)
    qwTh = _wprep(q_w)
    kvwT3 = kv_w.T.astype(np.float32).reshape(C, H, 2 * HD)
    kwTh = _wprep(kvwT3[:, :, :HD].reshape(C, C).T)
    vwTh = _wprep(kvwT3[:, :, HD:].reshape(C, C).T)
    pwTh = _wprep(proj_w)
    pbh = np.ascontiguousarray(proj_b.astype(np.float32).reshape(CT, P).T)
    in_maps = []
    for b in range(B):
        xb = x[b]                                   # [N, C]
        xTb = np.ascontiguousarray(
            xb.T.reshape(CT, P, N).transpose(1, 0, 2)).astype(bf)
        idxb = np.ascontiguousarray(
            key_ind[b].astype(np.int32).reshape(MCH, P).T)
        in_maps.append({
            "xT": xTb, "xbf": xb.astype(bf), "idx": idxb,
            "qwT": qwTh, "kwT": kwTh, "vwT": vwTh, "pwT": pwTh, "pb": pbh,
        })
    return in_maps


def kernel(x, key_ind, q_w, kv_w, proj_w, proj_b, _trace=False, _results=None):
    from concourse.bass_utils import run_bass_kernel_spmd

    nc = _get_nc()
    in_maps = _prep_core_inputs(x, key_ind, q_w, kv_w, proj_w, proj_b)
    res = run_bass_kernel_spmd(nc, in_maps, core_ids=list(range(B)), trace=_trace)
    if _results is not None:
        _results.append(res)
    out = np.empty((B, N, C), dtype=np.float32)
    for b in range(B):
        out[b] = res.results[b]["out"].astype(np.float32).reshape(C, N).T
    return out


# revision 67
# speedup vs baseline: 1.2141x; 1.0597x over previous
"""Sparse attention kernel for Trainium2 (8 NeuronCores, data-parallel over batch).

Reference computation (per batch row b):
    q    = x @ q_w.T                                  [N, C]
    xkv  = x[key_ind]                                 [NKV, C]
    kv   = xkv @ kv_w.T -> per-head k, v              [NKV, 2C]
    attn = softmax((q*scale) @ k.T) @ v               [N, C]
    out  = attn @ proj_w.T + proj_b                   [N, C]

Per-core layout (core = one batch row), all-bf16 matmul dataflow:
  - q/k computed feature-major (qT/kT [feat, tok]); scores ST [m, n] in PSUM;
    P = exp(ST*scale) written bf16 to SBUF by the Act engine.
  - PV is "flipped": stationary = P [m, n-128], moving = v_aug [m, 65]
    (65th column of ones gives the softmax denominator for free), so the
    PSUM output [n, 65] uses full 128 output partitions per matmul row.
  - attn normalized token-major (one broadcast multiply per 6 heads), then
    PE-transposed back to feature-major for the output projection.
  - Weights land as one large DMA each ([P, CT*C] layout prepped on host),
    spread across the SP and Act DMA queues; warmup matmuls burn the PE
    p-state ramp while they arrive.
"""
import sys

sys.path.insert(0, "/opt/trn_rl_repo")

import numpy as np  # noqa: E402

B, N, C = 8, 2048, 768
NKV = 512
H = 12
HD = C // H          # 64
SCALE = HD ** -0.5
P = 128
CT = C // P          # 6 feature tiles
NC2 = 512            # token chunk
NCH = N // NC2       # 4 chunks
MCH = NKV // P       # 4 key chunks
G = H // 2           # 6 head pairs

_CACHE = {}


def _build():
    import concourse.bass as bass
    import concourse.mybir as mybir
    import concourse.tile as tile
    from concourse import bacc
    from concourse.masks import make_identity
    from contextlib import ExitStack

    F32 = mybir.dt.float32
    BF = mybir.dt.bfloat16
    I32 = mybir.dt.int32
    Exp = mybir.ActivationFunctionType.Exp

    nc = bacc.Bacc("TRN2", target_bir_lowering=False, debug=False, num_devices=8)

    xT = nc.dram_tensor("xT", [P, CT, N], BF, kind="ExternalInput")
    xbf = nc.dram_tensor("xbf", [N, C], BF, kind="ExternalInput")
    idx = nc.dram_tensor("idx", [P, MCH], I32, kind="ExternalInput")
    qwT = nc.dram_tensor("qwT", [P, CT * C], BF, kind="ExternalInput")
    kwT = nc.dram_tensor("kwT", [P, CT * C], BF, kind="ExternalInput")
    vwT = nc.dram_tensor("vwT", [P, CT * C], BF, kind="ExternalInput")
    pwT = nc.dram_tensor("pwT", [P, CT * C], BF, kind="ExternalInput")
    pb = nc.dram_tensor("pb", [P, CT], F32, kind="ExternalInput")
    out = nc.dram_tensor("out", [CT, P, N], BF, kind="ExternalOutput")

    with tile.TileContext(nc) as tc, ExitStack() as top:
        const = top.enter_context(tc.tile_pool(name="const", bufs=1))
        xpool = top.enter_context(tc.tile_pool(name="xpool", bufs=1))
        qpool = top.enter_context(tc.tile_pool(name="qpool", bufs=2))
        ppool = top.enter_context(tc.tile_pool(name="ppool", bufs=2))
        npool = top.enter_context(tc.tile_pool(name="npool", bufs=1))
        tpool = top.enter_context(tc.tile_pool(name="tpool", bufs=1))
        opool = top.enter_context(tc.tile_pool(name="opool", bufs=1))
        rpool = top.enter_context(tc.tile_pool(name="rpool", bufs=2))

        # early PSUM pools: fresh banks so q/scores start during the prologue
        ps_q = top.enter_context(tc.tile_pool(name="ps_q", bufs=1, space="PSUM"))
        ps_st = top.enter_context(tc.tile_pool(name="ps_st", bufs=2, space="PSUM"))

        # ---------- warmup: burn the PE p-state ramp on dummies ----------
        zw = const.tile([P, NC2], BF, tag="zw")
        nc.vector.memset(zw[:], 0.0)
        wps = ps_q.tile([P, NC2], F32, tag="m2", name="wps")
        for w in range(14):
            nc.tensor.matmul(wps[:], zw[:, 0:P], zw[:],
                             start=(w == 0), stop=(w == 13))

        # ---------- DMAs: one large transfer per tensor, criticality order ----
        ph = top.enter_context(ExitStack())
        gpool = ph.enter_context(tc.tile_pool(name="kvph", bufs=1))
        idx_sb = const.tile([P, MCH], I32, tag="idx")
        nc.sync.dma_start(idx_sb[:], idx[:])
        xTc = {}
        t = xpool.tile([P, CT * NC2], BF, tag="xTc", name="xTc0")
        nc.sync.dma_start(t[:].rearrange("p (i n) -> p i n", n=NC2),
                          xT[:, :, 0:NC2])
        xTc[0] = t
        qwTb = const.tile([P, CT * C], BF, tag="qwTb")
        nc.sync.dma_start(qwTb[:], qwT[:])
        xkv_tiles = []
        for k in range(MCH):
            xkv = gpool.tile([P, C], BF, tag=f"xkv{k}")
            nc.gpsimd.indirect_dma_start(
                out=xkv[:], out_offset=None, in_=xbf[:],
                in_offset=bass.IndirectOffsetOnAxis(ap=idx_sb[:, k:k + 1], axis=0))
            xkv_tiles.append(xkv)
            if k == 0:
                # squeezed between gathers: ready before the first transpose
                ident = const.tile([P, P], BF, tag="ident")
                make_identity(nc, ident[:])
        # remaining inputs issue on the Pool queue with virtual timestamps
        # so the scheduler orders their descriptor-gens AFTER the gathers;
        # the in-order Pool queue then keeps their transfers behind them
        kwTb = gpool.tile([P, CT * C], BF, tag="kwTb")
        with tc.tile_wait_until(0.008):
            nc.gpsimd.dma_start(kwTb[:], kwT[:])
        vwTb = gpool.tile([P, CT * C], BF, tag="vwTb")
        with tc.tile_wait_until(0.010):
            nc.gpsimd.dma_start(vwTb[:], vwT[:])
        pwTb = const.tile([P, CT * C], BF, tag="pwTb")
        pb_sb = const.tile([P, CT], F32, tag="pb")
        with tc.tile_wait_until(0.013):
            nc.gpsimd.dma_start(pwTb[:], pwT[:])
            nc.gpsimd.dma_start(pb_sb[:], pb[:])

        qT = {}

        def emit_q(ch, j):
            qp = ps_q.tile([P, NC2], F32, tag="m2", name="qp")
            for i in range(CT):
                nc.tensor.matmul(qp[:], qwTb[:, i * C + j * P:i * C + (j + 1) * P],
                                 xTc[ch][:, i * NC2:(i + 1) * NC2],
                                 start=(i == 0), stop=(i == CT - 1))
            qt = qpool.tile([P, NC2], BF, tag=f"qT{j}")
            nc.vector.tensor_copy(qt[:], qp[:])
            qT[(ch, j)] = qt

        # q for chunk 0 is the PE's bridge work while the gathers land
        for j in range(CT):
            emit_q(0, j)

        tc.tile_set_cur_wait(0.011)

        # ---------- KV phase: k-staggered so PE work starts per gather ------
        ps_ktr = ph.enter_context(tc.tile_pool(name="ps_ktr", bufs=1, space="PSUM"))
        ps_kv = ph.enter_context(tc.tile_pool(name="ps_kv", bufs=2, space="PSUM"))

        xkvT = gpool.tile([P, CT * NKV], BF, tag="xkvT")
        xkvT3 = xkvT[:].rearrange("p (i m) -> p i m", m=NKV)
        kTb = const.tile([P, G * NKV], BF, tag="kTb")
        kTb3 = kTb[:].rearrange("p (g m) -> p g m", m=NKV)
        for k in range(MCH):
            xkv = xkv_tiles[k]
            tr1 = ps_ktr.tile([P, NC2], BF, tag="ktr")
            for i in range(4):
                nc.tensor.transpose(tr1[:, i * P:(i + 1) * P],
                                    xkv[:, i * P:(i + 1) * P], ident[:])
            tr2 = ps_ktr.tile([P, NC2], BF, tag="ktr")
            for i in range(4, CT):
                nc.tensor.transpose(tr2[:, (i - 4) * P:(i - 3) * P],
                                    xkv[:, i * P:(i + 1) * P], ident[:])
            nc.vector.tensor_copy(
                xkvT3[:, 0:4, k * P:(k + 1) * P],
                tr1[:].rearrange("p (i m) -> p i m", m=P))
            nc.vector.tensor_copy(
                xkvT3[:, 4:CT, k * P:(k + 1) * P],
                tr2[:, 0:2 * P].rearrange("p (i m) -> p i m", m=P))
            # kT for this key block: per pair g a [128-feat, 128-key] region,
            # accumulated over the 6 input-feature tiles
            kpA = ps_kv.tile([P, NC2], F32, tag="kv")
            kpB = ps_kv.tile([P, 2 * P], F32, tag="kv", name="kpB",
                             padded_shape=[P, NC2])
            for g in range(G):
                sl = kpA[:, g * P:(g + 1) * P] if g < 4 else \
                    kpB[:, (g - 4) * P:(g - 3) * P]
                for i in range(CT):
                    nc.tensor.matmul(
                        sl, kwTb[:, i * C + g * P:i * C + (g + 1) * P],
                        xkvT[:, i * NKV + k * P:i * NKV + (k + 1) * P],
                        start=(i == 0), stop=(i == CT - 1))
            nc.vector.tensor_copy(kTb3[:, 0:4, k * P:(k + 1) * P],
                                  kpA[:].rearrange("p (g m) -> p g m", m=P))
            nc.vector.tensor_copy(kTb3[:, 4:CT, k * P:(k + 1) * P],
                                  kpB[:].rearrange("p (g m) -> p g m", m=P))

        tc.tile_set_cur_wait(0.0)

        # ---------- main loop over token chunks ----------
        Pt = {}

        def emit_head_A(ch, h):
            g, par = h // 2, h % 2
            b0 = par * HD
            for kp in range(2):
                st = ps_st.tile([P, 2 * NC2], F32, tag="st")
                for k2 in range(2):
                    k = 2 * kp + k2
                    nc.tensor.matmul(
                        st[:, k2 * NC2:(k2 + 1) * NC2],
                        kTb[b0:b0 + HD, g * NKV + k * P:g * NKV + (k + 1) * P],
                        qT[(ch, g)][b0:b0 + HD, :], start=True, stop=True)
                pt = ppool.tile([P, 2 * NC2], BF, tag=f"P{h}_{kp}")
                nc.scalar.activation(pt[:], st[:], Exp, scale=SCALE)
                Pt[(ch, h, kp)] = pt

        # chunk 0's scores run standalone (nothing to overlap with yet)
        for h in range(H):
            emit_head_A(0, h)

        # v-projection is only needed by chunk 0's PV phase: emitting it
        # after the chunk-0 scores keeps the Act engine fed from ~13us
        vaug_sb = []
        for k in range(MCH):
            va = const.tile([P, H * (HD + 1)], BF, tag=f"vaug{k}")
            nc.vector.memset(va[:], 1.0)
            vaug_sb.append(va)
        for k in range(MCH):
            va3 = vaug_sb[k][:].rearrange("p (h x) -> p h x", x=HD + 1)
            for half in range(2):
                vp = ps_kv.tile([P, NC2], F32, tag="kv")
                for i in range(CT):
                    nc.tensor.matmul(
                        vp[:, 0:6 * HD],
                        xkvT[:, i * NKV + k * P:i * NKV + (k + 1) * P],
                        vwTb[:, i * C + half * 6 * HD:i * C + (half + 1) * 6 * HD],
                        start=(i == 0), stop=(i == CT - 1))
                nc.vector.tensor_copy(
                    va3[:, 6 * half:6 * half + 6, 0:HD],
                    vp[:, 0:6 * HD].rearrange("p (h x) -> p h x", x=HD))
        ph.close()

        # ---------- late PSUM pools (reuse released prologue banks) ----------
        ps_pv = top.enter_context(tc.tile_pool(name="ps_pv", bufs=2, space="PSUM"))
        ps_pj = top.enter_context(tc.tile_pool(name="ps_pj", bufs=1, space="PSUM"))

        def emit_xtc(chn):
            t = xpool.tile([P, CT * NC2], BF, tag="xTc", name=f"xTc{chn}")
            nc.sync.dma_start(
                t[:].rearrange("p (i n) -> p i n", n=NC2),
                xT[:, :, chn * NC2:(chn + 1) * NC2])
            xTc[chn] = t

        emit_xtc(1)

        for ch in range(NCH):
            base_n = ch * NC2
            # phase B: flipped PV + normalization + transpose + projection,
            # with the NEXT chunk's q chains, scores, and exps woven in so
            # the Act engine never starves while the PE works through B
            attnN = {}
            for s in range(4):
                attnN[s] = npool.tile([P, C], BF, tag=f"attnN{s}", name=f"attnN{s}")
            ojb = opool.tile([P, CT * NC2], BF, tag="ojb")
            ojb3 = ojb[:].rearrange("p (j n) -> p j n", n=NC2)

            seg = 0
            for pair in range(2):
                sset = (2 * pair, 2 * pair + 1)
                atTd = {}
                trd = {}
                for s in sset:
                    atTd[s] = tpool.tile([P, C], BF, tag=f"attnT{s}",
                                         name=f"atT{s}")
                # 2-head PV groups: group j's PV can start right after head
                # 2j+1's exp, and feeds transpose j immediately.  The last
                # chunk has no exp-gating (and nothing woven), so it uses
                # coarse 6-head groups to minimize chain hops in the drain.
                hpg = 6 if ch == NCH - 1 else 2
                jpg = hpg // 2
                for grp in range(H // hpg):
                    if ch + 1 < NCH:
                        if seg == 8 and ch + 2 < NCH:
                            emit_xtc(ch + 2)
                        if seg % 2 == 0:
                            emit_q(ch + 1, seg // 2)
                        emit_head_A(ch + 1, seg)
                    seg += 1
                    for s in sset:
                        pvt = ps_pv.tile([P, hpg * (HD + 1)], F32, tag="pv")
                        for h2 in range(hpg):
                            h = hpg * grp + h2
                            for k in range(MCH):
                                nc.tensor.matmul(
                                    pvt[:, h2 * (HD + 1):(h2 + 1) * (HD + 1)],
                                    Pt[(ch, h, k // 2)][
                                        :, (k % 2) * NC2 + s * P:
                                        (k % 2) * NC2 + (s + 1) * P],
                                    vaug_sb[k][:, h * (HD + 1):(h + 1) * (HD + 1)],
                                    start=(k == 0), stop=(k == MCH - 1))
                        # normalization: 1/denominator, then broadcast mult
                        pv3 = pvt[:].rearrange("p (h x) -> p h x", x=HD + 1)
                        rc = rpool.tile([P, hpg], F32, tag="rc",
                                        padded_shape=[P, 6])
                        rc3 = rc[:].rearrange("p (h x) -> p h x", x=1)
                        nc.vector.reciprocal(rc3, pv3[:, :, HD:HD + 1])
                        attnN3 = attnN[s][:].rearrange("p (h x) -> p h x", x=HD)
                        a0, a1 = bass.broadcast_tensor_aps(pv3[:, :, 0:HD], rc3)
                        nc.vector.tensor_tensor(
                            out=attnN3[:, hpg * grp:hpg * grp + hpg, :],
                            in0=a0, in1=a1, op=mybir.AluOpType.mult)
                        for jj in range(jpg):
                            j = grp * jpg + jj
                            if j % 3 == 0 and s == sset[0]:
                                # one wide tile per 3-group wave holds BOTH
                                # subtiles' transposes (no slot contention)
                                trd[0] = ps_pj.tile([P, CT * P], BF, tag="m1",
                                                    bufs=1, name=f"trw{j // 3}")
                            trt = trd[0]
                            toff = (s - sset[0]) * 3 * P
                            nc.tensor.transpose(
                                trt[:, toff + (j % 3) * P:toff + (j % 3 + 1) * P],
                                attnN[s][:, j * P:(j + 1) * P], ident[:])
                            if j % 3 == 2:
                                half = 3 * P * (j // 3)
                                nc.vector.tensor_copy(
                                    atTd[s][:, half:half + 3 * P],
                                    trt[:, toff:toff + 3 * P])
                # proj per subtile
                for s in sset:
                    atT = atTd[s]
                    pjA = ps_q.tile([P, NC2], F32, tag="m2", name="pjA")
                    pjB = ps_pj.tile([P, 2 * P], F32, tag="m1", name="pjB",
                                     padded_shape=[P, 3 * P])
                    for j in range(CT):
                        sl = pjA[:, j * P:(j + 1) * P] if j < 4 else \
                            pjB[:, (j - 4) * P:(j - 3) * P]
                        for i in range(CT):
                            nc.tensor.matmul(sl,
                                             pwTb[:, i * C + j * P:i * C + (j + 1) * P],
                                             atT[:, i * P:(i + 1) * P],
                                             start=(i == 0), stop=(i == CT - 1))
                    # batched bias add: psum -> oj staging, bias broadcast per j
                    pjA3 = pjA[:].rearrange("p (j n) -> p j n", n=P)
                    pbA = pb_sb[:, 0:4].rearrange("p (j x) -> p j x", x=1)
                    b0, b1 = bass.broadcast_tensor_aps(pjA3, pbA)
                    nc.vector.tensor_tensor(
                        out=ojb3[:, 0:4, s * P:(s + 1) * P],
                        in0=b0, in1=b1, op=mybir.AluOpType.add)
                    pjB3 = pjB[:].rearrange("p (j n) -> p j n", n=P)
                    pbB = pb_sb[:, 4:CT].rearrange("p (j x) -> p j x", x=1)
                    c0, c1 = bass.broadcast_tensor_aps(pjB3, pbB)
                    nc.vector.tensor_tensor(
                        out=ojb3[:, 4:CT, s * P:(s + 1) * P],
                        in0=c0, in1=c1, op=mybir.AluOpType.add)
                # drain this half-chunk's output; the very last one goes in
                # two j-half pieces so the tail transfer is half as long
                hb = pair * 2 * P
                if ch == NCH - 1 and pair == 1:
                    for jh in range(2):
                        nc.sync.dma_start(
                            out[3 * jh:3 * jh + 3, :,
                                base_n + hb:base_n + hb + 2 * P].rearrange(
                                "j p n -> p j n"),
                            ojb3[:, 3 * jh:3 * jh + 3, hb:hb + 2 * P])
                else:
                    nc.sync.dma_start(
                        out[:, :, base_n + hb:base_n + hb + 2 * P].rearrange(
                            "j p n -> p j n"),
                        ojb3[:, :, hb:hb + 2 * P])

    nc.compile()
    return nc


def _get_nc():
    if "nc" not in _CACHE:
        _CACHE["nc"] = _build()
    return _CACHE["nc"]


def _wprep(w):
    """[C_out, C_in] weight -> transposed, partition-major [P, CT*C]."""
    import ml_dtypes
    wt = np.ascontiguousarray(
        w.T.astype(np.float32).reshape(CT, P, C).transpose(1, 0, 2).reshape(P, CT * C))
    return wt.astype(ml_dtypes.bfloat16)


def _prep_core_inputs(x, key_ind, q_w, kv_w, proj_w, proj_b):
    """Build the 8 per-core input maps (host-side prep, all bf16)."""
    import ml_dtypes
    bf = ml_dtypes.bfloat16
    x = np.ascontiguousarray(x, dtype=np.float32)
    qwTh = _wprep(q_w)
    kvwT3 = kv_w.T.astype(np.float32).reshape(C, H, 2 * HD)
    kwTh = _wprep(kvwT3[:, :, :HD].reshape(C, C).T)
    vwTh = _wprep(kvwT3[:, :, HD:].reshape(C, C).T)
    pwTh = _wprep(proj_w)
    pbh = np.ascontiguousarray(proj_b.astype(np.float32).reshape(CT, P).T)
    in_maps = []
    for b in range(B):
        xb = x[b]                                   # [N, C]
        xTb = np.ascontiguousarray(
            xb.T.reshape(CT, P, N).transpose(1, 0, 2)).astype(bf)
        idxb = np.ascontiguousarray(
            key_ind[b].astype(np.int32).reshape(MCH, P).T)
        in_maps.append({
            "xT": xTb, "xbf": xb.astype(bf), "idx": idxb,
            "qwT": qwTh, "kwT": kwTh, "vwT": vwTh, "pwT": pwTh, "pb": pbh,
        })
    return in_maps


def kernel(x, key_ind, q_w, kv_w, proj_w, proj_b, _trace=False, _results=None):
    from concourse.bass_utils import run_bass_kernel_spmd

    nc = _get_nc()
    in_maps = _prep_core_inputs(x, key_ind, q_w, kv_w, proj_w, proj_b)
    res = run_bass_kernel_spmd(nc, in_maps, core_ids=list(range(B)), trace=_trace)
    if _results is not None:
        _results.append(res)
    out = np.empty((B, N, C), dtype=np.float32)
    for b in range(B):
        out[b] = res.results[b]["out"].astype(np.float32).reshape(C, N).T
    return out
